# revision 40
# baseline (speedup 1.0000x reference)
"""Contextual loss kernel for Trainium2 (Bass/Tile), 8 NeuronCores.

Reference computation (per batch b, B=4, C=128, N=64*64=4096):
  mean_y[c] = spatial mean of feature_y
  fx,fy centered by mean_y; columns L2-normalized over channels
  S[n,m]    = <fxn[:,n], fyn[:,m]>           (cosine similarity)
  d = 1-S;  d_norm = d / (min_m d + 1e-3);  w = exp((1-d_norm)/h);  A = w/sum_m w
  CX[b] = mean_n max_m A;  loss = -log(CX)

Per-row identity used on device (with Smax = max_m S, c = 1/(h*(1-Smax+eps))):
  max_m A = 1 / sum_m exp(c*(S[m]-Smax))

x is centered but NOT normalized on device: with T = <xc_n, ycn_m> and
invx_n = 1/||xc_n||, S = invx*T, so Smax = invx*Tmax and the exp becomes
exp(ss*T + bb) with per-row ss = invx/(h*(1+eps-invx*Tmax)), bb = -ss*Tmax
-- both fold into the ACT activation's per-partition scale/bias, removing a
full elementwise pass over x.

Sharding: 8 cores = 4 batches x 2 row-halves. Each core gets its half of
feature_x's rows ([2048,128]) plus the full feature_y ([4096,128]) of its
batch, computes sum_rows 1/r locally; host combines and takes -log.

Main loop: two interleaved matmul passes with recompute (PE streams both;
the second matmul is cheaper than any PSUM->SBUF staging; GpSimd has no
PSUM port so only DVE can row-max and only ACT can exp the PSUM quarters).
Pass 1 row-maxes each [P,1024] PSUM quarter on DVE; the tiny per-row chain
(ss/bb) runs on GpSimd (block 0 all-DVE to start the exp stream early).
Pass 2 re-matmuls and ACT exp's each quarter in place with accum_out row
sums; it trails pass 1 by one iteration, emitted first each iteration to
keep ACT fed. Steady state: ACT ~5.2us/block (bound, ~97% duty), DVE ~4.9,
PE ~3.4+. Preprocessing is phase-ordered on disjoint engine queues with
the y-column pipeline (the main loop's gate) prioritized, and the column
-sum tree + mean run behind the input DMA.
"""

import numpy as np

import concourse.bacc as bacc
import concourse.bass as bass
import concourse.tile as tile
from concourse import masks, mybir
from concourse.bass_utils import run_bass_kernel_spmd

F32 = mybir.dt.float32
F32R = mybir.dt.float32r
AF = mybir.ActivationFunctionType
OP = mybir.AluOpType

B = 4
C = 128
N = 4096          # spatial positions per batch
ROWS = N // 2     # rows of S per core (x-half)
P = 128           # partitions
NYT = N // P      # 32 y tiles
NXT = ROWS // P   # 16 x tiles
CHUNK = 512       # matmul free dim (one PSUM bank)
QUART = 1024      # columns per PSUM quarter (2 banks)
NQ = N // QUART   # 4 quarters per row block
NRB = ROWS // P   # 16 row blocks per core
GRP = 8           # tiles per preprocessing group

H_PARAM = 0.1
EPS_MIN = 0.001
EPS_NORM = 1e-10


def build_nc():
    nc = bacc.Bacc(None)
    fx = nc.declare_dram_parameter("fx", [ROWS, C], F32, isOutput=False)
    fy = nc.declare_dram_parameter("fy", [N, C], F32, isOutput=False)
    part = nc.declare_dram_parameter("part", [P, 1], F32, isOutput=True)

    fy_t = fy.rearrange("(i p) c -> p i c", p=P)   # [128, 32, 128]
    fx_t = fx.rearrange("(i p) c -> p i c", p=P)   # [128, 16, 128]

    with tile.TileContext(nc) as tc:
        with (
            tc.tile_pool(name="singles", bufs=1) as singles,
            tc.tile_pool(name="raw", bufs=1) as raw,
            tc.tile_pool(name="tmats", bufs=1) as tmats,
            tc.tile_pool(name="stat", bufs=3) as stat,
            tc.tile_pool(name="scratch", bufs=2) as scratch,
        ):
            # ---- constants ----
            identity = singles.tile([P, P], F32)
            masks.make_identity(nc, identity[:])
            ones_col = singles.tile([P, 1], F32)
            nc.vector.memset(ones_col[:], 1.0)
            ones_row = singles.tile([1, P], F32)
            nc.vector.memset(ones_row[:], 1.0)
            # warm the ACT table with a sqrt-capable set (contains Square
            # too) so no table reload lands mid-prep; Exp loads once at the
            # start of the main loop.
            warm = singles.tile([P, 1], F32)
            nc.scalar.activation(out=warm[:], in_=ones_col[:], func=AF.Sqrt)

            ns_y = singles.tile([P, NYT], F32)      # y squared norms
            std_y = singles.tile([P, NYT], F32)
            inv_y = singles.tile([P, NYT], F32)
            ns_x = singles.tile([P, NXT], F32)
            std_x = singles.tile([P, NXT], F32)
            invx = singles.tile([P, NXT], F32)      # 1/||xc_n||, [P, rb]
            mean_sb = singles.tile([1, C], F32)
            colsum = singles.tile([P, C], F32)
            mx_all = singles.tile([P, NRB], F32)    # Tmax per row
            ss_all = singles.tile([P, NRB], F32)    # exp scale = c*invx
            bb_all = singles.tile([P, NRB], F32)    # exp bias = -ss*Tmax
            invr_all = singles.tile([P, NRB], F32)

            # ---- load inputs (fewer dma_starts: queue config is ~0.6us
            # of serial SP-sequencer time apiece) ----
            ysp = raw.tile([P, NYT, C], F32)   # y, spatial-major tiles
            xsp = raw.tile([P, NXT, C], F32)
            for j in range(8):
                nc.sync.dma_start(
                    out=ysp[:, j * 4:(j + 1) * 4, :],
                    in_=fy_t[:, j * 4:(j + 1) * 4, :],
                )
            for j in range(4):
                nc.sync.dma_start(
                    out=xsp[:, j * 4:(j + 1) * 4, :],
                    in_=fx_t[:, j * 4:(j + 1) * 4, :],
                )

            # Flat views: DVE/Pool pay per-inner-segment overhead on 3D APs
            # (~2x on [P,8,128]); all elementwise prep runs on [P, G*C] flats.
            GC = GRP * C
            ysp_f = ysp[:].rearrange("p i c -> p (i c)")
            xsp_f = xsp[:].rearrange("p i c -> p (i c)")

            # ---- y column sums: pair-tree adds chunk-by-chunk behind the
            # DMA (each add waits only its two chunks) ----
            HC = 4 * C   # one DMA chunk, flattened
            t4 = singles.tile([P, 4, HC], F32)
            t4f = t4[:].rearrange("p i c -> p (i c)")
            for j in range(4):
                nc.vector.tensor_add(t4f[:, j * HC:(j + 1) * HC],
                                     ysp_f[:, 2 * j * HC:(2 * j + 1) * HC],
                                     ysp_f[:, (2 * j + 1) * HC:
                                             (2 * j + 2) * HC])
            acc = singles.tile([P, HC], F32)
            nc.vector.tensor_add(acc[:], t4f[:, 0:HC], t4f[:, HC:2 * HC])
            nc.vector.tensor_add(acc[:], acc[:], t4f[:, 2 * HC:3 * HC])
            nc.vector.tensor_add(acc[:], acc[:], t4f[:, 3 * HC:4 * HC])
            nc.vector.reduce_sum(
                colsum[:], acc[:].rearrange("p (i c) -> p c i", i=4),
                axis=mybir.AxisListType.X,
            )
            # mean over spatial, broadcast to [P, C] (f32 K=1 matmuls are
            # 4 cyc/row -- keep the broadcast narrow)
            mean_bc = singles.tile([P, C], F32)
            with tc.tile_pool(name="ps_mean", bufs=1,
                              space=bass.MemorySpace.PSUM) as psm:
                ps_mean = psm.tile([1, C], F32)
                nc.tensor.matmul(ps_mean[:], ones_col[:], colsum[:],
                                 start=True, stop=True)
                nc.scalar.mul(mean_sb[:], ps_mean[:], 1.0 / N)
            with tc.tile_pool(name="ps_bc", bufs=1,
                              space=bass.MemorySpace.PSUM) as psb:
                ps_bc = psb.tile([P, C], F32)
                nc.tensor.matmul(ps_bc[:], ones_row[:], mean_sb[:],
                                 start=True, stop=True)
                nc.vector.tensor_copy(mean_bc[:], ps_bc[:])
            mean_g = mean_bc[:].rearrange("p (u c) -> p u c", u=1)

            # ---- preprocess + transpose, PHASE-ordered ----
            # Per-engine queues are in-order; interleaving dependent stages
            # of different groups lets one slow chain block later independent
            # work. Emit phase by phase (all centers, all squares, ...), y
            # before x (the main loop's first gate is the full ytc set).
            ytc = [tmats.tile([P, CHUNK], F32R, tag=f"ytc{j}", name=f"ytc{j}")
                   for j in range(N // CHUNK)]     # y chunks, channel-major
            xt = tmats.tile([P, ROWS], F32R)       # x, channel-major

            def yflat(g):
                return ysp_f[:, g * GC:(g + 1) * GC]

            def xflat(g):
                return xsp_f[:, g * GC:(g + 1) * GC]

            def ymean(g):
                return mean_g.broadcast_to([P, GRP, C])

            def yview(g):
                return ysp[:, g * GRP:(g + 1) * GRP, :]

            def xview(g):
                return xsp[:, g * GRP:(g + 1) * GRP, :]

            NYG, NXG = NYT // GRP, NXT // GRP
            # centers: disjoint engine queues; x-group0 first on DVE (it
            # gates the main loop's lhs for blocks 0-7 and block-0's invx)
            nc.vector.tensor_sub(yview(0), yview(0), ymean(0))
            nc.vector.tensor_sub(yview(1), yview(1), ymean(1))
            nc.gpsimd.tensor_sub(yview(2), yview(2), ymean(2))
            nc.gpsimd.tensor_sub(yview(3), yview(3), ymean(3))
            nc.vector.tensor_sub(xview(0), xview(0), ymean(0))
            nc.gpsimd.tensor_sub(xview(1), xview(1), ymean(1))
            # y squares first (ACT), then per-group norms/inv in DVE order
            sqs = []
            for g in range(NYG):
                sq = scratch.tile([P, GC], F32, tag=f"sq{g % 2}")
                nc.scalar.activation(out=sq[:], in_=yflat(g), func=AF.Square)
                sqs.append(sq)
            for g in range(NYG):
                nc.vector.reduce_sum(
                    ns_y[:, g * GRP:(g + 1) * GRP],
                    sqs[g][:].rearrange("p (t c) -> p t c", t=GRP),
                    axis=mybir.AxisListType.X)
                nc.scalar.activation(out=std_y[:, g * GRP:(g + 1) * GRP],
                                     in_=ns_y[:, g * GRP:(g + 1) * GRP],
                                     func=AF.Sqrt)
                nc.vector.reciprocal(inv_y[:, g * GRP:(g + 1) * GRP],
                                     std_y[:, g * GRP:(g + 1) * GRP])
            # scales (strided bcast multiply, split DVE/Pool)
            for g in range(NYG):
                eng = nc.vector if g < 2 else nc.gpsimd
                ig = inv_y[:, g * GRP:(g + 1) * GRP].rearrange(
                    "p (t u) -> p t u", u=1)
                eng.tensor_mul(yview(g), yview(g),
                               ig.broadcast_to([P, GRP, C]))
            # x stats (x itself is never scaled)
            xsqs = []
            for g in range(NXG):
                sq = scratch.tile([P, GC], F32, tag=f"sq{g % 2}")
                nc.scalar.activation(out=sq[:], in_=xflat(g), func=AF.Square)
                xsqs.append(sq)
            for g in range(NXG):
                nc.vector.reduce_sum(
                    ns_x[:, g * GRP:(g + 1) * GRP],
                    xsqs[g][:].rearrange("p (t c) -> p t c", t=GRP),
                    axis=mybir.AxisListType.X)
                nc.scalar.activation(out=std_x[:, g * GRP:(g + 1) * GRP],
                                     in_=ns_x[:, g * GRP:(g + 1) * GRP],
                                     func=AF.Sqrt)
                nc.vector.reciprocal(invx[:, g * GRP:(g + 1) * GRP],
                                     std_x[:, g * GRP:(g + 1) * GRP])

            # ---- main loop helpers: pass1 (PE+DVE+Pool), pass2 (PE+ACT) --
            def p1_quarter(rb, q, pool, mxq):
                lhs = xt[:, rb * P:(rb + 1) * P]
                ps = pool.tile([P, QUART], F32, tag="p1", name="ps1")
                for j in range(2):
                    nc.tensor.matmul(
                        ps[:, j * CHUNK:(j + 1) * CHUNK],
                        lhs, ytc[2 * q + j][:],
                        start=True, stop=True)
                nc.vector.reduce_max(mxq[:, q:q + 1], ps[:],
                                     axis=mybir.AxisListType.X)

            def p1_chain(rb, mxq):
                # per-row chain: tiny [P,1] ops on Pool (parallel to DVE's
                # maxes; on DVE the scheduler interleaves them behind the
                # next block's 1.1us maxes, stalling the exp stream).
                # (Pool is SBUF-only; divide/max aren't tensor_tensor ops.)
                eng = nc.gpsimd
                mxs = mx_all[:, rb:rb + 1]
                nc.vector.reduce_max(mxs, mxq[:], axis=mybir.AxisListType.X)
                smax = stat.tile([P, 1], F32, tag="smax")
                eng.tensor_mul(smax[:], mxs, invx[:, rb:rb + 1])
                hd = stat.tile([P, 1], F32, tag="hd")
                eng.tensor_scalar(
                    out=hd[:], in0=smax[:], scalar1=1.0 + EPS_MIN,
                    scalar2=-H_PARAM, op0=OP.subtract, op1=OP.mult)
                ihd = stat.tile([P, 1], F32, tag="ihd")
                nc.vector.reciprocal(ihd[:], hd[:])
                eng.tensor_mul(ss_all[:, rb:rb + 1], ihd[:],
                               invx[:, rb:rb + 1])
                eng.tensor_scalar(
                    out=bb_all[:, rb:rb + 1], in0=ss_all[:, rb:rb + 1],
                    scalar1=mxs, scalar2=-1.0, op0=OP.mult, op1=OP.mult)

            def pass1(rb, pool):
                mxq = stat.tile([P, NQ], F32, tag="mxq", name="mxq")
                for q in range(NQ):
                    p1_quarter(rb, q, pool, mxq)
                p1_chain(rb, mxq)

            def pass2(rb, pool):
                lhs = xt[:, rb * P:(rb + 1) * P]
                rq = stat.tile([P, NQ], F32, tag="rq", name="rq")
                for q in range(NQ):
                    ps = pool.tile([P, QUART], F32, tag="p2", name="ps2")
                    for j in range(2):
                        nc.tensor.matmul(
                            ps[:, j * CHUNK:(j + 1) * CHUNK],
                            lhs, ytc[2 * q + j][:],
                            start=True, stop=True)
                    nc.scalar.activation(
                        out=ps[:], in_=ps[:], func=AF.Exp,
                        bias=bb_all[:, rb:rb + 1], scale=ss_all[:, rb:rb + 1],
                        accum_out=rq[:, q:q + 1])
                rs = stat.tile([P, 1], F32, tag="rs")
                nc.vector.reduce_sum(rs[:], rq[:], axis=mybir.AxisListType.X)
                nc.vector.reciprocal(invr_all[:, rb:rb + 1], rs[:])

            with tc.tile_pool(name="ps_p1", bufs=2,
                              space=bass.MemorySpace.PSUM) as pool1:
                # Transposes (PE) + PSUM->SBUF cast copies (split ACT/DVE),
                # with block 0's pass-1 quarters interleaved: PE's in-order
                # queue would otherwise hold ALL main matmuls behind the
                # last y transpose (gated by the last scale), idling DVE.
                # ps_tr (4x1 bank) + pool1 (2x2 banks) = 8 banks exactly.
                with tc.tile_pool(name="ps_tr", bufs=4,
                                  space=bass.MemorySpace.PSUM) as ps_tr:
                    def ybatch(b2):
                        pst = ps_tr.tile([P, 4 * P], F32, tag="pst")
                        for k in range(4):
                            t = b2 * 4 + k
                            nc.tensor.transpose(pst[:, k * P:(k + 1) * P],
                                                ysp[:, t, :], identity[:])
                        if b2 % 2 == 0:
                            nc.scalar.copy(ytc[b2][:], pst[:])
                        else:
                            nc.vector.tensor_copy(ytc[b2][:], pst[:])

                    def xbatch(b2):
                        pst = ps_tr.tile([P, 4 * P], F32, tag="pst")
                        for k in range(4):
                            t = b2 * 4 + k
                            nc.tensor.transpose(pst[:, k * P:(k + 1) * P],
                                                xsp[:, t, :], identity[:])
                        x0 = b2 * 4 * P
                        if b2 % 2 == 0:
                            nc.scalar.copy(xt[:, x0:x0 + 4 * P], pst[:])
                        else:
                            nc.vector.tensor_copy(xt[:, x0:x0 + 4 * P],
                                                  pst[:])

                    # All y batches before block-0's quarters: interleaving
                    # them put the late batch copies BEHIND the q0/q1 maxes
                    # in DVE's in-order queue, delaying q3 (measured).
                    for b2 in range(8):
                        ybatch(b2)
                    for b2 in range(2):
                        xbatch(b2)
                    mxq0 = stat.tile([P, NQ], F32, tag="mxq", name="mxq")
                    for q in range(NQ):
                        p1_quarter(0, q, pool1, mxq0)
                    p1_chain(0, mxq0)
                    for b2 in range(2, 4):
                        xbatch(b2)

                # ACT's prep queue is done: switch the table to an
                # Exp-capable set now, off the first-exp path
                nc.scalar.activation(out=warm[:], in_=ones_col[:],
                                     func=AF.Exp)

                with tc.tile_pool(name="ps_p2", bufs=2,
                                  space=bass.MemorySpace.PSUM) as pool2:
                    # pass2 trails pass1 by ONE iteration (chain(rb) is done
                    # well before the ACT stream reaches block rb); emitted
                    # first each iteration to keep ACT fed.
                    for it in range(1, NRB + 1):
                        pass2(it - 1, pool2)
                        if it < NRB:
                            pass1(it, pool1)

            # ---- reduce row contributions, write out ----
            part_sb = singles.tile([P, 1], F32)
            nc.vector.reduce_sum(part_sb[:], invr_all[:],
                                 axis=mybir.AxisListType.X)
            nc.sync.dma_start(out=part[:], in_=part_sb[:])

    nc.compile()
    return nc


_NC_CACHE = None


def _get_nc():
    global _NC_CACHE
    if _NC_CACHE is None:
        _NC_CACHE = build_nc()
    return _NC_CACHE


def _in_maps(feature_x, feature_y):
    fx = np.ascontiguousarray(
        np.asarray(feature_x, dtype=np.float32).reshape(B, N, C))
    fy = np.ascontiguousarray(
        np.asarray(feature_y, dtype=np.float32).reshape(B, N, C))
    maps = []
    for core in range(8):
        b, h = divmod(core, 2)
        maps.append({
            "fx": np.ascontiguousarray(fx[b, h * ROWS:(h + 1) * ROWS, :]),
            "fy": fy[b],
        })
    return maps


def _combine(results):
    sums = [float(np.asarray(r["part"], dtype=np.float64).sum())
            for r in results]
    loss = np.empty(B, dtype=np.float64)
    for b in range(B):
        cx = (sums[2 * b] + sums[2 * b + 1]) / N
        loss[b] = -np.log(cx)
    return loss.astype(np.float32)


def kernel(feature_x, feature_y):
    nc = _get_nc()
    res = run_bass_kernel_spmd(nc, _in_maps(feature_x, feature_y),
                               core_ids=list(range(8)))
    return _combine(res.results)


def kernel_traced(feature_x, feature_y, **kwargs):
    """Like kernel() but with tracing; returns (loss, BassKernelResults)."""
    nc = _get_nc()
    res = run_bass_kernel_spmd(nc, _in_maps(feature_x, feature_y),
                               core_ids=list(range(8)), trace=True, **kwargs)
    return _combine(res.results), res
